# revision 65
# baseline (speedup 1.0000x reference)
"""CRF loss on 8 TRN2 cores — slab-streamed chunk-parallel forward recursion.

Sharding: pure data parallel, 256 batch rows -> 8 cores x 32 rows.

Denominator (log-partition): the 1024-step forward recursion runs as C=32
concurrent chunks of 32 payload steps (chunk (tau,k) covers steps
tau*128+k*32+[0,32)), fused into a [97, 1024] state, two groups of 512
(tau 0-3 / 4-7), state column = tau*128 + b*4 + k.  Each chunk gets a
4-step burn-in from a uniform vector using the last 4 steps of the
previous chunk (Perron-Frobenius mixing ~2e-7, far below bf16 state
noise).  Emissions are staged in 8 SP-MAJOR SLABS (slab sigma = steps
4*sigma..4*sigma+3 of every chunk), with slab 7 (the burn-in source)
staged FIRST so the recursion starts after ~2 slabs and then streams
concurrently with the remaining staging — phase A and phase B fully
overlap.  Per slot: one bf16 matmul per group (stationary W =
exp(transitions)) into PSUM, then a DVE multiply by the slot's exp'd
emissions.  Renorm every 8 slots: ones-matmul colsums -> DVE reciprocal
-> ACT.Ln(sv) accumulated to a scalar (only sum_b logZ_b is needed, so
ALL per-column log bookkeeping collapses into ACT accum_out scalars) ->
PE ones-outer-product broadcast -> the scale is folded into the XC slice
consumed LAG slots later (off the critical chain).  Chunk stitching:
  sum_col logZ = sum ln A - sum ln B - sum_events sum ln sv,
  B==1 for chunk 0 (exact restart), A end-weighted for chunk 31.

Numerator: emission scores x[b,s,tag] are gathered by GPSIMD
indirect_copy straight from the raw f32 staging slabs (per 16-partition
group the wrapped index list is shared, so each partition gathers all 16
partners' picks from its own row and a static diagonal-block mask STT
selects + accumulates its own 32).  Transition/start/end scores use
host-built COUNT matrices (pure tag bookkeeping): score = <Count, T>
computed by one tiny STT with accum_out.
"""

import numpy as np

import concourse.bacc as bacc
import concourse.bass as bass
import concourse.mybir as mybir
import concourse.tile as tile
from concourse import bass_utils, masks

B, S, T = 256, 1024, 97
NCORES = 8
BL = B // NCORES          # 32 batch rows per core
C = 32                    # chunks
ELL = S // C              # 32 payload steps per chunk
BETA = 4                  # burn-in steps
NSLOT = ELL + BETA        # 36
NSLAB = 8                 # staging slabs
SPS = ELL // NSLAB        # 4 steps (per chunk) per slab
CB = C * BL               # 1024 fused state columns
HG = CB // 2              # 512 per group
REN_SLOTS = [8, 16, 24]   # renorm events (fold applied LAG later)
LAG = 3
SLABF = NSLAB * SPS * T   # 3104 free elements per slab staging tile
NEM = 512                 # gathered emission values per slab (16 partners x 32)

F32 = mybir.dt.float32
BF16 = mybir.dt.bfloat16
U16 = mybir.dt.uint16
ALU = mybir.AluOpType
AXX = mybir.AxisListType
ACT = mybir.ActivationFunctionType

# lnS strip layout
LN_A_CA, LN_A_CB, LN_A_C31, LN_A_W, LN_B_CA, LN_B_CB, LN_B_C0 = range(7)
LN_EV0 = 7                # events 7,8,9
NLN = 12


def build_module():
    nc = bacc.Bacc("TRN2", target_bir_lowering=False, debug=False)

    x_d = nc.dram_tensor("x_d", [BL, S, T], F32, kind="ExternalInput").ap()
    trans_d = nc.dram_tensor("trans_d", [T, T], F32, kind="ExternalInput").ap()
    se_d = nc.dram_tensor("se_d", [T, 2], F32, kind="ExternalInput").ap()
    cnt_d = nc.dram_tensor("cnt_d", [T, T], F32, kind="ExternalInput").ap()
    c0l_d = nc.dram_tensor("c0l_d", [T, 2], F32, kind="ExternalInput").ap()
    widx_d = nc.dram_tensor("widx_d", [128, NSLAB * 32], U16,
                            kind="ExternalInput").ap()
    pmod_d = nc.dram_tensor("pmod_d", [128, 1], F32, kind="ExternalInput").ap()
    num_d = nc.dram_tensor("num_d", [1, 16], F32, kind="ExternalOutput").ap()
    lns_d = nc.dram_tensor("lns_d", [1, NLN], F32, kind="ExternalOutput").ap()

    # x viewed so one slab is a 4-level AP with 1552B contiguous runs:
    # s = tau*128 + k*32 + 4*sigma + i; (i,j) merges into one 388-elem run
    xv = x_d.rearrange("b (tau k s4 i) j -> b k tau s4 (i j)", tau=NSLAB,
                       k=4, i=SPS)

    with tile.TileContext(nc) as tc:
        with (
            tc.tile_pool(name="const", bufs=1) as const_pool,
            tc.tile_pool(name="stage", bufs=1) as stage_pool,
            tc.tile_pool(name="eg", bufs=1) as eg_pool,
            tc.tile_pool(name="ea", bufs=2) as ea_pool,
            tc.tile_pool(name="eb", bufs=2) as eb_pool,
            tc.tile_pool(name="sv", bufs=2) as sv_pool,
            tc.tile_pool(name="lnj", bufs=1) as lnj_pool,
            tc.tile_pool(name="tp", bufs=2, space=bass.MemorySpace.PSUM) as tp_pool,
            tc.tile_pool(name="pa", bufs=1, space=bass.MemorySpace.PSUM) as pa_pool,
            tc.tile_pool(name="pb", bufs=1, space=bass.MemorySpace.PSUM) as pb_pool,
            tc.tile_pool(name="cs", bufs=2, space=bass.MemorySpace.PSUM) as cs_pool,
        ):
            # ---------------- critical-path-first input DMAs ----------------
            # slab 7 feeds burn-in slot 0 — its 4 k-DMAs go absolutely
            # first; the small param DMAs follow (W is only needed ~5us
            # later)
            ident = const_pool.tile([128, 128], F32)
            masks.make_identity(nc, ident[:])

            ones_col = const_pool.tile([T, 1], BF16)
            nc.vector.memset(ones_col[:], 1.0)
            ones_row = const_pool.tile([1, T], BF16)
            nc.vector.memset(ones_row[:], 1.0)
            ones128 = const_pool.tile([128, 1], F32)
            nc.vector.memset(ones128[:], 1.0)

            # c//32 plane for the emission diagonal-block mask
            iotaC = const_pool.tile([128, NEM], F32)
            nc.gpsimd.iota(iotaC[:], pattern=[[1, 16], [0, 32]], base=0,
                           channel_multiplier=0,
                           allow_small_or_imprecise_dtypes=True)

            naccS = const_pool.tile([128, 16], F32)
            nc.vector.memset(naccS[:], 0.0)
            lnS = const_pool.tile([1, NLN], F32)
            nc.vector.memset(lnS[:], 0.0)
            dumpT = const_pool.tile([T, T], F32)
            dumpE = const_pool.tile([128, NEM], F32)

            # XC: exp'd emissions, flat col = sigma*4096 + tau*512 + i*128
            # + b*4 + k
            XC = const_pool.tile([T, S * BL], BF16)
            XCv = XC[:].rearrange("p (sg tau i b k) -> p sg tau i b k",
                                  sg=NSLAB, tau=NSLAB, i=SPS, b=BL)

            # ---------------- slab machinery ----------------
            slab_stage = {}

            def slab_dma(sg):
                stg = stage_pool.tile([128, SLABF], F32, tag=f"stg{sg % 4}")
                stgk = stg[:].rearrange("(b k) f -> b k f", k=4)
                for kk in range(4):
                    nc.sync.dma_start(stgk[:, kk, :], xv[:, kk, :, sg, :])
                slab_stage[sg] = stg

            def slab_piece(sg, i):
                """Transpose+exp step-index i of slab sg across ALL 8 taus
                (8 transposes, one 2-bank PSUM tile, one strided ACT exp
                into XC).  i-major pieces mean recursion slot (sg, i) waits
                on exactly ONE piece's exp instead of the whole slab."""
                stg = slab_stage[sg]
                bank = tp_pool.tile([T, 1024], F32, tag="tp")
                for tau in range(8):
                    nc.tensor.transpose(
                        bank[:, tau * 128:(tau + 1) * 128],
                        stg[:, (tau * SPS + i) * T:(tau * SPS + i) * T + T],
                        ident[:])
                nc.scalar.activation(
                    XCv[:, sg, :, i, :, :],
                    bank[:].rearrange("p (t b k) -> p t b k", t=NSLAB, k=4),
                    ACT.Exp)

            slab_eg = {}

            def slab_gather(sg):
                """Numerator emission gather for slab sg (Pool; depends only
                on the raw staged tile, so it can run well before the slab's
                transposes)."""
                stg = slab_stage[sg]
                egath = eg_pool.tile([128, NEM], F32, tag=f"eg{(sg + 1) % 8 % 3}")
                nc.gpsimd.indirect_copy(
                    egath[:], stg[:], widx[:, sg * 32:(sg + 1) * 32], True)
                slab_eg[sg] = egath

            def slab_stt(sg):
                """Mask-select + accumulate slab sg's own emissions (DVE);
                emitted a few slots after the gather so it never head-of-line
                blocks the recursion TTs behind an in-flight Pool gather."""
                nc.vector.scalar_tensor_tensor(
                    dumpE[:], iotaC[:], pmod[:], slab_eg[sg][:],
                    ALU.is_equal, ALU.mult,
                    accum_out=naccS[:, sg:sg + 1])

            # ---------------- pre-loop: all slab DMAs upfront ----------
            # (dedicated SBUF per slab: zero buffer-reuse waits; slab 7
            # first since burn-in consumes it)
            slab_dma(7)
            tr_stage = const_pool.tile([T, T], F32)
            nc.sync.dma_start(tr_stage[:], trans_d[:, :])
            se_stage = const_pool.tile([T, 2], F32)
            nc.sync.dma_start(se_stage[:], se_d[:, :])
            W = const_pool.tile([T, T], BF16)
            nc.scalar.activation(W[:], tr_stage[:], ACT.Exp)
            exp_start = const_pool.tile([T, 1], F32)
            nc.scalar.activation(exp_start[:], se_stage[:, 0:1], ACT.Exp)
            exp_end = const_pool.tile([T, 1], F32)
            nc.scalar.activation(exp_end[:], se_stage[:, 1:2], ACT.Exp)
            slab_dma(0)
            # remaining small inputs after the startup-critical slabs
            cnt = const_pool.tile([T, T], F32)
            nc.sync.dma_start(cnt[:], cnt_d[:, :])
            c0l = const_pool.tile([T, 2], F32)
            nc.sync.dma_start(c0l[:], c0l_d[:, :])
            widx = const_pool.tile([128, NSLAB * 32], U16)
            nc.sync.dma_start(widx[:], widx_d[:, :])
            pmod = const_pool.tile([128, 1], F32)
            nc.sync.dma_start(pmod[:], pmod_d[:, :])
            for sg in [1, 2, 3, 4, 5, 6]:
                slab_dma(sg)
            slab_gather(7)
            slab_gather(0)

            # numerator transition/start/end scores from count matrices
            nc.vector.scalar_tensor_tensor(
                dumpT[:], cnt[:], 1.0, tr_stage[:], ALU.mult, ALU.mult,
                accum_out=naccS[0:T, 8:9])
            nc.vector.scalar_tensor_tensor(
                dumpT[:, 0:2], c0l[:], 1.0, se_stage[:], ALU.mult, ALU.mult,
                accum_out=naccS[0:T, 9:10])

            # pre-loop: slabs 7+0 fully transposed/exp'd back-to-back so
            # PE streams transposes continuously (stays ramped); slabs 1-6
            # stream inside the loop with multi-slot margins
            for t2 in range(4):
                slab_piece(7, t2)
            for t2 in range(4):
                slab_piece(0, t2)

            # ---------------- recursion ----------------
            eA = ea_pool.tile([T, HG], BF16, tag="eA")
            nc.vector.memset(eA[:], 1.0 / T)
            eB = eb_pool.tile([T, HG], BF16, tag="eB")
            nc.vector.memset(eB[:], 1.0 / T)

            def colsums(ea_t, eb_t):
                csA = cs_pool.tile([1, HG], F32, tag="ev")
                nc.tensor.matmul(csA[:], ones_col[:], ea_t[:])
                csB = cs_pool.tile([1, HG], F32, tag="ev")
                nc.tensor.matmul(csB[:], ones_col[:], eb_t[:])
                return csA, csB

            def ln_accum(src_ap, slot, scale=1.0):
                """ACT.Ln of src (any AP shape) with scalar free-sum into
                lnS[slot]; the Ln values themselves go to scratch.  scale
                is an exact power of two folded in before the Ln (the HW Ln
                only covers roughly [1e-19, 1e18]); the host adds the
                compensating n*ln(scale) back."""
                jt = lnj_pool.tile([1, len(REN_SLOTS) * CB], F32, tag="lnj")
                out = jt[:, 0:src_ap.free_size()]
                if len(src_ap.shape) > 2:
                    pat = "p (" + " ".join(f"d{i}" for i in
                                           range(len(src_ap.shape) - 1)) + ") -> p " + \
                          " ".join(f"d{i}" for i in range(len(src_ap.shape) - 1))
                    kw = {f"d{i}": src_ap.shape[1 + i]
                          for i in range(len(src_ap.shape) - 1)}
                    out = out.rearrange(pat, **kw)
                nc.scalar.activation(out, src_ap, ACT.Ln, scale=scale,
                                     accum_out=lnS[:, slot:slot + 1])

            # B colsums and event reciprocals are kept in SBUF so every
            # ACT.Ln runs at the very end (2 act-func-set loads total)
            bkeep = const_pool.tile([1, CB], F32)
            # bf16 scales: exact-logged (Ln reads the same bf16 values the
            # fold applies) and the PE broadcast runs at 1 cycle/row
            svkeep = const_pool.tile([1, len(REN_SLOTS) * CB], BF16)

            pend_fold = {}
            ev_idx = 0
            for s in range(NSLOT):
                if s == BETA:
                    # B-capture: colsum of v_{BETA-1} -> SBUF for later Ln
                    # (DVE copy; burn-in slots leave DVE mostly idle)
                    csA, csB = colsums(eA, eB)
                    nc.vector.tensor_copy(bkeep[:, 0:HG], csA[:])
                    nc.vector.tensor_copy(bkeep[:, HG:CB], csB[:])

                if s in REN_SLOTS:
                    csA, csB = colsums(eA, eB)
                    sv = svkeep[:, ev_idx * CB:(ev_idx + 1) * CB]
                    with nc.allow_low_precision(
                            reason="renorm scale is bf16 by design; the "
                                   "applied scale is ln-logged exactly"):
                        nc.vector.reciprocal(sv[:, 0:HG], csA[:])
                        nc.vector.reciprocal(sv[:, HG:CB], csB[:])
                    ev_idx += 1
                    svbcA = cs_pool.tile([T, HG], F32, tag="ev")
                    nc.tensor.matmul(svbcA[:], ones_row[:], sv[:, 0:HG])
                    svbcB = cs_pool.tile([T, HG], F32, tag="ev")
                    nc.tensor.matmul(svbcB[:], ones_row[:], sv[:, HG:CB])
                    pend_fold[s + LAG] = (svbcA, svbcB)

                if s in pend_fold:
                    svbcA, svbcB = pend_fold.pop(s)
                    sp = s - BETA
                    sg, i = sp // SPS, sp % SPS
                    xa = XCv[:, sg, 0:4, i, :, :]
                    nc.vector.tensor_tensor(
                        xa, xa, svbcA[:].rearrange("p (t b k) -> p t b k",
                                                   t=4, k=4), ALU.mult)
                    xb = XCv[:, sg, 4:8, i, :, :]
                    nc.vector.tensor_tensor(
                        xb, xb, svbcB[:].rearrange("p (t b k) -> p t b k",
                                                   t=4, k=4), ALU.mult)

                PA = pa_pool.tile([T, HG], F32, tag="PA")
                nc.tensor.matmul(PA[:], W[:], eA[:])
                PB = pb_pool.tile([T, HG], F32, tag="PB")
                nc.tensor.matmul(PB[:], W[:], eB[:])

                eA_new = ea_pool.tile([T, HG], BF16, tag="eA")
                eB_new = eb_pool.tile([T, HG], BF16, tag="eB")
                PAr = PA[:].rearrange("p (t b k) -> p t b k", t=4, k=4)
                PBr = PB[:].rearrange("p (t b k) -> p t b k", t=4, k=4)
                eAr = eA_new[:].rearrange("p (t b k) -> p t b k", t=4, k=4)
                eBr = eB_new[:].rearrange("p (t b k) -> p t b k", t=4, k=4)
                if s < BETA:
                    X7 = XCv[:, 7, :, s, :, :]  # [p, tau(8), b, k]
                    # chunks with k>=1 source own tau, k-1
                    nc.vector.tensor_tensor(
                        eAr[:, :, :, 1:4], PAr[:, :, :, 1:4],
                        X7[:, 0:4, :, 0:3], ALU.mult)
                    nc.vector.tensor_tensor(
                        eBr[:, :, :, 1:4], PBr[:, :, :, 1:4],
                        X7[:, 4:8, :, 0:3], ALU.mult)
                    # k=0 chunks source tau-1, k=3
                    nc.vector.tensor_tensor(
                        eAr[:, 1:4, :, 0:1], PAr[:, 1:4, :, 0:1],
                        X7[:, 0:3, :, 3:4], ALU.mult)
                    nc.vector.tensor_tensor(
                        eBr[:, 0:4, :, 0:1], PBr[:, 0:4, :, 0:1],
                        X7[:, 3:7, :, 3:4], ALU.mult)
                    # chunk 0: wrapped source tau 7, k=3
                    nc.vector.tensor_tensor(
                        eAr[:, 0:1, :, 0:1], PAr[:, 0:1, :, 0:1],
                        X7[:, 7:8, :, 3:4], ALU.mult)
                else:
                    sp = s - BETA
                    sg, i = sp // SPS, sp % SPS
                    nc.vector.tensor_tensor(
                        eAr[:, :, :, :], PAr[:, :, :, :],
                        XCv[:, sg, 0:4, i, :, :], ALU.mult)
                    nc.vector.tensor_tensor(
                        eBr[:, :, :, :], PBr[:, :, :, :],
                        XCv[:, sg, 4:8, i, :, :], ALU.mult)

                if s == BETA:
                    # chunk 0 exact restart: E_0 = exp(start) * X(step 0)
                    nc.vector.tensor_scalar_mul(
                        eAr[:, 0:1, :, 0:1], XCv[:, 0, 0:1, 0, :, 0:1],
                        exp_start[:])

                eA, eB = eA_new, eB_new

                # slab pipeline: one whole slab (32 transposes) per 4
                # slots, emitted as one clump so PE ramps to full clock
                # mid-burst instead of paying the MID-pstate tax on
                # scattered 8-transpose groups.  Pool gathers are issued as
                # slabs arrive; the DVE mask-STTs run ~6 slots after their
                # gather so neither blocks the recursion stream.
                if s < 24 and s % 4 == 0:
                    sg = s // 4 + 1
                    for t2 in range(4):
                        slab_piece(sg, t2)
                if s == 0:
                    slab_gather(1)
                elif s == 3:
                    slab_gather(2)
                elif s in (7, 11, 15, 19):
                    slab_gather((s - 7) // 4 + 3)
                stt_sched = {5: 7, 7: 0, 9: 1, 11: 2, 13: 3, 17: 4, 21: 5,
                             25: 6}
                if s in stt_sched:
                    slab_stt(stt_sched[s])

                if s == 27:
                    # B/event Ln bookkeeping overlapped with the last slots.
                    # A dummy Exp (same func set as the stream) reads slab
                    # 6's final exp output and writes the shared jt scratch;
                    # the WAW chain through jt then pins the Ln batch (and
                    # its one func-set switch) AFTER every emission Exp, so
                    # the scheduler cannot thrash the act-func table
                    # mid-stream.  svkeep leads (event-24 dependency).
                    pin = lnj_pool.tile([1, len(REN_SLOTS) * CB], F32,
                                        tag="lnj")
                    nc.scalar.activation(
                        pin[:, 0:128].rearrange("p (b k) -> p b k", k=4),
                        XCv[0:1, 6, 7, 3, :, :], ACT.Exp)
                    ln_accum(svkeep[:, :], LN_EV0, scale=2.0 ** 32)
                    ln_accum(bkeep[:, 0:HG], LN_B_CA)
                    ln_accum(bkeep[:, HG:CB], LN_B_CB)
                    c0ap = bkeep[:, 0:128].rearrange("p (b k) -> p b k",
                                                     k=4)[:, :, 0:1]
                    ln_accum(c0ap, LN_B_C0)


            # ---------------- A-capture + all Ln bookkeeping ----------------
            # NOTE: the csA/csB Ln readers MUST be emitted before csw
            # reuses a bank from the same PSUM pool (pool realloc assumes
            # the previous tile's readers were already emitted).
            # ACT.Ln cannot read PSUM on HW (garbage + poisons the ACT
            # accumulator) — bounce every colsum through SBUF first.
            csA, csB = colsums(eA, eB)
            akeep = const_pool.tile([1, CB], F32)
            nc.vector.tensor_copy(akeep[:, 0:HG], csA[:])
            nc.vector.tensor_copy(akeep[:, HG:CB], csB[:])
            wv = const_pool.tile([T, BL], F32)
            nc.vector.tensor_scalar_mul(
                wv[:], eB[:].rearrange("p (t b k) -> p t b k",
                                       t=4, k=4)[:, 3:4, :, 3:4], exp_end[:])
            wvb = const_pool.tile([T, BL], BF16)
            nc.vector.tensor_copy(wvb[:], wv[:])
            csw = cs_pool.tile([1, BL], F32, tag="ev")
            nc.tensor.matmul(csw[:], ones_col[:], wvb[:])
            wkeep = const_pool.tile([1, BL], F32)
            nc.vector.tensor_copy(wkeep[:], csw[:])
            ln_accum(akeep[:, 0:HG], LN_A_CA, scale=2.0 ** -64)
            ln_accum(akeep[:, HG:CB], LN_A_CB, scale=2.0 ** -64)
            # chunk 31 (tau7 -> local t 3, k=3): cols 3*128 + b*4 + 3
            c31 = akeep[:, HG:CB].rearrange("p (t b k) -> p t b k",
                                            t=4, k=4)[:, 3:4, :, 3:4]
            ln_accum(c31, LN_A_C31, scale=2.0 ** -64)
            ln_accum(wkeep[:, :], LN_A_W, scale=2.0 ** -64)

            # ---------------- outputs ----------------
            nm = cs_pool.tile([1, 16], F32, tag="ev")
            nc.tensor.matmul(nm[:], ones128[:], naccS[:])
            nms = const_pool.tile([1, 16], F32)
            nc.vector.tensor_copy(nms[:], nm[:])
            nc.sync.dma_start(num_d[:, :], nms[:])
            nc.sync.dma_start(lns_d[:, :], lnS[:])

    nc.compile()
    return nc


_cached = {}


def _prep_core_inputs(inputs, tags, transitions, start, end, c):
    sl = slice(c * BL, (c + 1) * BL)
    tg = tags[sl].astype(np.int64)  # (BL, S)

    cnt = np.zeros((T, T), np.float32)
    np.add.at(cnt, (tg[:, :-1].ravel(), tg[:, 1:].ravel()), 1.0)
    c0l = np.zeros((T, 2), np.float32)
    np.add.at(c0l[:, 0], tg[:, 0], 1.0)
    np.add.at(c0l[:, 1], tg[:, -1], 1.0)

    # emission gather indices: per slab sg, group g (16 partitions = batch
    # rows 4g..4g+3 x k 0..3), wrapped list flat[c] = partner (q=c//32)'s
    # pick for (tau=(c%32)//4, i=(c%32)%4):
    #   idx = tau*388 + i*97 + tags[b_q, tau*128 + k_q*32 + 4*sg + i]
    widx = np.zeros((128, NSLAB * 32), np.uint16)
    cc = np.arange(NEM)
    q, m = cc // 32, cc % 32
    tau, i = m // SPS, m % SPS
    for sg in range(NSLAB):
        for g in range(8):
            bq, kq = 4 * g + q // 4, q % 4
            steps = tau * 128 + kq * 32 + 4 * sg + i
            idx = (tau * 388 + i * T + tg[bq, steps]).astype(np.uint16)
            widx[16 * g:16 * (g + 1), sg * 32:(sg + 1) * 32] = \
                idx.reshape(32, 16).T
    pmod = (np.arange(128) // 16 * 0 + np.arange(128) % 16)[:, None]

    return {
        "x_d": np.ascontiguousarray(inputs[sl]),
        "trans_d": transitions,
        "se_d": np.ascontiguousarray(np.stack([start, end], axis=1)),
        "cnt_d": cnt,
        "c0l_d": c0l,
        "widx_d": widx,
        "pmod_d": np.ascontiguousarray(pmod.astype(np.float32)),
    }


def kernel(inputs, transitions, start_transitions, end_transitions, tags, mask):
    inputs = np.ascontiguousarray(np.asarray(inputs, dtype=np.float32))
    tags = np.ascontiguousarray(np.asarray(tags, dtype=np.int32))
    transitions = np.ascontiguousarray(np.asarray(transitions, dtype=np.float32))
    start = np.asarray(start_transitions, dtype=np.float32)
    end = np.asarray(end_transitions, dtype=np.float32)

    if "nc" not in _cached:
        _cached["nc"] = build_module()
    nc = _cached["nc"]

    in_maps = [
        _prep_core_inputs(inputs, tags, transitions, start, end, c)
        for c in range(NCORES)
    ]
    res = bass_utils.run_bass_kernel_spmd(nc, in_maps,
                                          core_ids=list(range(NCORES)))
    _cached["last_results"] = res

    loss = np.float64(0.0)
    for c in range(NCORES):
        out = res.results[c]
        num = out["num_d"][0].astype(np.float64)
        ln = out["lns_d"][0].astype(np.float64)
        numerator = num[0:8].sum() + num[8] + num[9]
        LN2 = np.log(2.0)
        lnA = (ln[LN_A_CA] + ln[LN_A_CB] - ln[LN_A_C31] + ln[LN_A_W]
               + (HG + HG - BL + BL) * 64 * LN2)
        lnB = ln[LN_B_CA] + ln[LN_B_CB] - ln[LN_B_C0]
        lnEv = ln[LN_EV0] - len(REN_SLOTS) * CB * 32 * LN2
        logzsum = lnA - lnB - lnEv
        loss += numerator - logzsum
    return np.float32(loss)
